# revision 1
# baseline (speedup 1.0000x reference)
"""Trainium2 Bass kernel for per-edge dot products (DGL u_dot_v).

score[e] = sum_d h[src[e], d] * h[dst[e], d]   for 640K edges, 10K nodes, D=128.

Strategy (8 NeuronCores, data-parallel over edges):
  - Each core gets 80K edges; h stays in HBM, replicated per core.
  - Per tile of 4096 edges: two HBM-source `dma_gather`s pull h rows as
    contiguous descriptors into SBUF as [128 edges, 32, 128 features]
    (edge i -> partition i%128, slot i//128), spread across SWDGE queues.
  - VectorE: one elementwise multiply + one free-dim `tensor_reduce` per tile
    produce [128, 32] fp32 scores.
  - One contiguous DMA writes [128, 625] scores out; the host inverts the
    (partition, chunk) interleave with a transpose-reshape.
"""

import sys

import numpy as np

for _p in ("/opt/trn_rl_repo", "/opt/pypackages"):
    if _p not in sys.path:
        sys.path.append(_p)

import concourse.mybir as mybir  # noqa: E402
import concourse.tile as tile  # noqa: E402
from concourse import bacc  # noqa: E402
from concourse.bass_utils import run_bass_kernel_spmd  # noqa: E402

N_NODES = 10000
D_FEAT = 128
N_EDGES = 640000
N_CORES = 8
E_PER = N_EDGES // N_CORES  # 80000
TILE_E = 4096  # edges per gather tile

# Gather precision: fp32 h rows are exact; fp16 halves gather traffic at
# ~1.2e-4 scale-relative error (products/accumulation stay fp32 on DVE).
GATHER_DTYPE = "f32"
N_QUEUES = 4  # SWDGE queues to spread gathers over (1..4)

_BUILT = {}


def _edge_tiles(e_per):
    tiles = []
    s = 0
    while s < e_per:
        t = min(TILE_E, e_per - s)
        assert t % 128 == 0
        tiles.append((s, t))
        s += t
    return tiles


def build(e_per=E_PER, reps=1, gdt=None, n_queues=None):
    """Build + compile the per-core Bass program (cached).

    reps > 1 repeats the whole compute (for wall-clock differencing in the
    bench harness); output is identical for every rep."""
    gdt = gdt or GATHER_DTYPE
    n_queues = n_queues or N_QUEUES
    key = (e_per, reps, gdt, n_queues)
    if key in _BUILT:
        return _BUILT[key]

    i16 = mybir.dt.int16
    f32 = mybir.dt.float32
    gdtype = f32 if gdt == "f32" else mybir.dt.float16

    nc = bacc.Bacc(
        "TRN2", target_bir_lowering=False, debug=False, num_swdge_queues=n_queues
    )

    h_d = nc.dram_tensor("h", [N_NODES, D_FEAT], gdtype, kind="ExternalInput")
    srcw_d = nc.dram_tensor("srcw", [128, e_per // 16], i16, kind="ExternalInput")
    dstw_d = nc.dram_tensor("dstw", [128, e_per // 16], i16, kind="ExternalInput")
    out_d = nc.dram_tensor("scores", [128, e_per // 128], f32, kind="ExternalOutput")

    with tile.TileContext(nc) as tc:
        with (
            tc.tile_pool(name="const", bufs=1) as constp,
            tc.tile_pool(name="gather", bufs=3) as gpool,
            tc.tile_pool(name="prod", bufs=2) as ppool,
            tc.tile_pool(name="outp", bufs=1) as outp,
        ):
            srcw = constp.tile([128, e_per // 16], i16)
            dstw = constp.tile([128, e_per // 16], i16)
            scores = outp.tile([128, e_per // 128], f32)

            nc.sync.dma_start(srcw[:], srcw_d[:])
            nc.sync.dma_start(dstw[:], dstw_d[:])

            q = 0
            for start, t in _edge_tiles(e_per) * reps:
                nchunk = t // 128
                hu = gpool.tile([128, nchunk, D_FEAT], gdtype, tag="hu")
                hv = gpool.tile([128, nchunk, D_FEAT], gdtype, tag="hv")
                for dst_t, idx_t in ((hu, srcw), (hv, dstw)):
                    nc.gpsimd.dma_gather(
                        dst_t[:],
                        h_d[:],
                        idx_t[:, start // 16 : (start + t) // 16],
                        num_idxs=t,
                        num_idxs_reg=t,
                        elem_size=D_FEAT,
                        single_packet=False,
                        queue_num=q % n_queues,
                    )
                    q += 1
                prod = ppool.tile([128, nchunk, D_FEAT], f32)
                nc.vector.tensor_mul(prod[:], hu[:], hv[:])
                nc.vector.tensor_reduce(
                    scores[:, start // 128 : start // 128 + nchunk],
                    prod[:],
                    axis=mybir.AxisListType.X,
                    op=mybir.AluOpType.add,
                )

            nc.sync.dma_start(out_d[:], scores[:])

    nc.compile()
    _BUILT[key] = nc
    return nc


def wrap_idx(ix):
    """Edge indices [E_c] -> int16 [128, E_c/16]: slot j read from
    (partition j%16, col j//16), replicated across the 8 GPSIMD core groups."""
    w = ix.astype(np.int16).reshape(-1, 16).T  # [16, E_c/16]
    return np.ascontiguousarray(np.tile(w, (8, 1)))


# ---------------------------------------------------------------------------
# Paired variant: sort edges by src so pairs of edges share one src-row gather
# (hu descriptors halve: 160K -> 129K rows gathered per core).
# Device slot layout: blocks of 256 slots = 128 pairs; pair i -> slots
# (i//128)*256 + i%128 + {0, 128}, so both edges of a pair sit on partition
# i%128, matching the hu2 gather interleave (row i -> partition i%128).
# ---------------------------------------------------------------------------

E2_PER = 86016  # padded device slots per core (multiple of 256, >= worst pad)


def build_paired(e2=E2_PER, reps=1, n_queues=None):
    n_queues = n_queues or N_QUEUES
    key = ("paired", e2, reps, n_queues)
    if key in _BUILT:
        return _BUILT[key]

    i16 = mybir.dt.int16
    f32 = mybir.dt.float32

    nc = bacc.Bacc(
        "TRN2", target_bir_lowering=False, debug=False, num_swdge_queues=n_queues
    )

    h_d = nc.dram_tensor("h", [N_NODES, D_FEAT], f32, kind="ExternalInput")
    srcw_d = nc.dram_tensor("srcw", [128, e2 // 32], i16, kind="ExternalInput")
    dstw_d = nc.dram_tensor("dstw", [128, e2 // 16], i16, kind="ExternalInput")
    out_d = nc.dram_tensor("scores", [128, e2 // 128], f32, kind="ExternalOutput")

    with tile.TileContext(nc) as tc:
        with (
            tc.tile_pool(name="const", bufs=1) as constp,
            tc.tile_pool(name="gather", bufs=3) as gpool,
            tc.tile_pool(name="prod", bufs=2) as ppool,
            tc.tile_pool(name="outp", bufs=4) as outp,
        ):
            srcw = constp.tile([128, e2 // 32], i16)
            dstw = constp.tile([128, e2 // 16], i16)

            nc.sync.dma_start(srcw[:], srcw_d[:])
            nc.sync.dma_start(dstw[:], dstw_d[:])

            q = 0
            for start, t in _edge_tiles(e2) * reps:
                nb = t // 256  # pair-blocks in this tile
                hu2 = gpool.tile([128, nb, D_FEAT], f32, tag="hu2")
                hv3 = gpool.tile([128, t // 128, D_FEAT], f32, tag="hv3")
                prod3 = ppool.tile([128, t // 128, D_FEAT], f32)
                hv4 = hv3[:].rearrange("p (b r) f -> p b r f", r=2)
                prod4 = prod3[:].rearrange("p (b r) f -> p b r f", r=2)
                for hf in range(2):
                    p0 = start // 2 + hf * (t // 4)
                    nc.gpsimd.dma_gather(
                        hu2[:, hf * nb // 2 : (hf + 1) * nb // 2, :],
                        h_d[:],
                        srcw[:, p0 // 16 : (p0 + t // 4) // 16],
                        num_idxs=t // 4,
                        num_idxs_reg=t // 4,
                        elem_size=D_FEAT,
                        single_packet=False,
                        queue_num=q % n_queues,
                    )
                    q += 1
                    h0 = hf * (t // 2)
                    nc.gpsimd.dma_gather(
                        hv3[:, hf * (t // 256) : (hf + 1) * (t // 256), :],
                        h_d[:],
                        dstw[:, (start + h0) // 16 : (start + h0 + t // 2) // 16],
                        num_idxs=t // 2,
                        num_idxs_reg=t // 2,
                        elem_size=D_FEAT,
                        single_packet=False,
                        queue_num=q % n_queues,
                    )
                    q += 1
                    bs = slice(hf * nb // 2, (hf + 1) * nb // 2)
                    nc.vector.tensor_mul(
                        prod4[:, bs, 0, :], hu2[:, bs, :], hv4[:, bs, 0, :]
                    )
                    nc.vector.tensor_mul(
                        prod4[:, bs, 1, :], hu2[:, bs, :], hv4[:, bs, 1, :]
                    )
                sc = outp.tile([128, t // 128], f32, tag="sc")
                nc.vector.tensor_reduce(
                    sc[:],
                    prod3[:],
                    axis=mybir.AxisListType.X,
                    op=mybir.AluOpType.add,
                )
                nc.sync.dma_start(
                    out_d[:, start // 128 : start // 128 + t // 128], sc[:]
                )

    nc.compile()
    _BUILT[key] = nc
    return nc


def prep_paired(s, d, e2=E2_PER):
    """Sort a core's edges by src, pad equal-src runs to even length, and lay
    pairs out in the device block order. Returns (hu_idx [e2/2], hv_idx [e2],
    ed_map [e2] original-edge-or--1) or None if padding overflows e2."""
    n = len(s)
    order = np.argsort(s, kind="stable")
    ss, dd = s[order], d[order]
    change = np.flatnonzero(np.diff(ss)) + 1
    starts = np.concatenate(([0], change))
    ends = np.concatenate((change, [n]))
    lens = ends - starts
    odd = (lens % 2).astype(bool)
    if n + int(odd.sum()) > e2:
        return None
    pads_before = np.concatenate(([0], np.cumsum(odd)[:-1]))
    new_pos = np.arange(n) + np.repeat(pads_before, lens)
    psrc = np.zeros(e2, np.int64)
    pdst = np.zeros(e2, np.int64)
    pedge = np.full(e2, -1, np.int64)
    psrc[new_pos] = ss
    pdst[new_pos] = dd
    pedge[new_pos] = order
    pad_slots = (ends + pads_before)[odd]
    psrc[pad_slots] = ss[ends[odd] - 1]
    j = np.arange(e2)
    ps = 2 * ((j // 256) * 128 + (j % 128)) + (j % 256) // 128
    return psrc[0::2], pdst[ps], pedge[ps]


def _kernel_flat(h, src, dst):
    """Unpaired path: one gather per edge endpoint."""
    nc = build(E_PER)
    in_maps = []
    for k in range(N_CORES):
        sl = slice(k * E_PER, (k + 1) * E_PER)
        in_maps.append(
            {"h": h, "srcw": wrap_idx(src[sl]), "dstw": wrap_idx(dst[sl])}
        )
    res = run_bass_kernel_spmd(nc, in_maps, list(range(N_CORES)))
    parts = []
    for k in range(N_CORES):
        sc = res.results[k]["scores"]  # [128, E_PER/128]; edge j at [j%128, j//128]
        parts.append(sc.T.reshape(-1))
    return np.concatenate(parts).astype(np.float32).reshape(N_EDGES, 1)


def kernel(h, src, dst):
    np_gdt = np.float32 if GATHER_DTYPE == "f32" else np.float16
    h = np.ascontiguousarray(np.asarray(h, dtype=np.float32).astype(np_gdt))
    src = np.asarray(src).astype(np.int64)
    dst = np.asarray(dst).astype(np.int64)

    preps = []
    for k in range(N_CORES):
        sl = slice(k * E_PER, (k + 1) * E_PER)
        preps.append(prep_paired(src[sl], dst[sl]))
    if any(p is None for p in preps):
        return _kernel_flat(h, src, dst)

    nc = build_paired(E2_PER)
    in_maps = []
    for hu_idx, hv_idx, _ in preps:
        in_maps.append(
            {"h": h, "srcw": wrap_idx(hu_idx), "dstw": wrap_idx(hv_idx)}
        )
    res = run_bass_kernel_spmd(nc, in_maps, list(range(N_CORES)))

    out = np.empty(N_EDGES, np.float32)
    for k in range(N_CORES):
        sc = res.results[k]["scores"]  # [128, E2/128]; device slot j at [j%128, j//128]
        flat = sc.T.reshape(-1)
        ed_map = preps[k][2]
        valid = ed_map >= 0
        out_local = np.empty(E_PER, np.float32)
        out_local[ed_map[valid]] = flat[valid]
        out[k * E_PER : (k + 1) * E_PER] = out_local
    return out.reshape(N_EDGES, 1)



# revision 5
# speedup vs baseline: 12.7930x; 12.7930x over previous
"""Trainium2 Bass kernel for per-edge dot products (DGL u_dot_v).

score[e] = sum_d h[src[e], d] * h[dst[e], d]   for 640K edges, 10K nodes, D=128.

Strategy (8 NeuronCores, data-parallel over edges; h replicated per core):
  - h is uploaded as a transposed packed-pair fp16 table hT2 [128, 10000, 2]:
    partition p holds the feature pair (2*(p%64), 2*(p%64)+1) of every node,
    duplicated across the two partition halves.  It lives in SBUF
    (40KB/partition), so all per-edge gathers are SBUF->SBUF on the GPSIMD
    engine via `ap_gather` (d=2: each index moves 4B x 16 partitions).  No
    SWDGE descriptors and no random HBM reads - that was the baseline's
    bottleneck (~60ns/descriptor on real HW).
  - The 8 GPSIMD core-groups get independent index streams: groups 0-3
    (partitions 0-63 = feature pairs 0-63) gather edge-stream A, groups 4-7
    gather edge-stream B, so one ap_gather of num_idxs=N covers 2N edges.
  - Per tile of 20480 edges (A=first half, B=second half): 2 ap_gathers
    (src/dst), one in-place DVE fp16 multiply, then the feature reduction
    runs on PE: onesAB[128,2].T @ prod (two strided matmuls accumulating the
    packed-pair halves) -> PSUM [2, 512] = 512 A-scores + 512 B-scores fp32.
  - PSUM chunks [2, 2048] DMA straight to HBM; the host inverts only the
    tile-level A/B interleave with one cheap reshape.
"""

import sys

import numpy as np

for _p in ("/opt/trn_rl_repo", "/opt/pypackages"):
    if _p not in sys.path:
        sys.path.append(_p)

import concourse.mybir as mybir  # noqa: E402
import concourse.tile as tile  # noqa: E402
from concourse import bacc  # noqa: E402
from concourse.bass_utils import run_bass_kernel_spmd  # noqa: E402

N_NODES = 10000
D_FEAT = 128
N_EDGES = 640000
N_CORES = 8
E_PER = N_EDGES // N_CORES  # 80000
E_DEV = 81920  # padded per-core edge count (4 tiles of 20480)
TILE_E = 20480  # edges per tile (2 streams of N_IDX)
N_IDX = TILE_E // 2  # ap_gather num_idxs per call
N_TILES = E_DEV // TILE_E  # 4
MM_N = 512  # edges per matmul pair (one PSUM bank row)
PS_CHUNK = 2048  # edge-columns per PSUM tile / output DMA row

_BUILT = {}


def build(reps=1):
    key = ("v1c", reps)
    if key in _BUILT:
        return _BUILT[key]

    i16 = mybir.dt.int16
    f16 = mybir.dt.float16
    f32 = mybir.dt.float32

    nc = bacc.Bacc("TRN2", target_bir_lowering=False, debug=False)

    hT2_d = nc.dram_tensor("hT2", [128, N_NODES, 2], f16, kind="ExternalInput")
    srcw_d = nc.dram_tensor("srcw", [128, E_DEV // 32], i16, kind="ExternalInput")
    dstw_d = nc.dram_tensor("dstw", [128, E_DEV // 32], i16, kind="ExternalInput")
    out_d = nc.dram_tensor(
        "scores", [N_TILES * (N_IDX // PS_CHUNK), 2, PS_CHUNK], f32,
        kind="ExternalOutput",
    )

    with tile.TileContext(nc) as tc:
        with (
            tc.tile_pool(name="const", bufs=1) as constp,
            tc.tile_pool(name="hu", bufs=2) as hupool,
            tc.tile_pool(name="hv", bufs=1) as hvpool,
            tc.tile_pool(name="sc", bufs=2) as scpool,
            tc.tile_pool(name="psum", bufs=2, space="PSUM") as pspool,
        ):
            hT2 = constp.tile([128, N_NODES, 2], f16)
            srcw = constp.tile([128, E_DEV // 32], i16)
            dstw = constp.tile([128, E_DEV // 32], i16)
            onesAB = constp.tile([128, 2], f16)

            nc.sync.dma_start(hT2[:], hT2_d[:])
            nc.sync.dma_start(srcw[:], srcw_d[:])
            nc.sync.dma_start(dstw[:], dstw_d[:])
            nc.vector.memset(onesAB[:], 0.0)
            nc.vector.memset(onesAB[0:64, 0:1], 1.0)
            nc.vector.memset(onesAB[64:128, 1:2], 1.0)

            for rep in range(reps):
                for t in range(N_TILES):
                    c0 = t * (N_IDX // 16)
                    c1 = (t + 1) * (N_IDX // 16)
                    hu = hupool.tile([128, N_IDX, 2], f16, tag="hu")
                    hv = hvpool.tile([128, N_IDX, 2], f16, tag="hv")
                    for dst_t, idx_t in ((hu, srcw), (hv, dstw)):
                        nc.gpsimd.ap_gather(
                            dst_t[:],
                            hT2[:],
                            idx_t[:, c0:c1],
                            channels=128,
                            num_elems=N_NODES,
                            d=2,
                            num_idxs=N_IDX,
                        )
                    # prod = hu * hv, in place into hu's buffer
                    nc.vector.tensor_mul(hu[:], hu[:], hv[:])
                    for c in range(N_IDX // PS_CHUNK):
                        ps = pspool.tile([2, PS_CHUNK], f32, tag="ps")
                        for j in range(PS_CHUNK // MM_N):
                            col = c * PS_CHUNK + j * MM_N
                            sl = ps[:, j * MM_N : (j + 1) * MM_N]
                            nc.tensor.matmul(
                                sl, onesAB[:], hu[:, col : col + MM_N, 0],
                                start=True, stop=False,
                            )
                            nc.tensor.matmul(
                                sl, onesAB[:], hu[:, col : col + MM_N, 1],
                                start=False, stop=True,
                            )
                        row = t * (N_IDX // PS_CHUNK) + c
                        sc = scpool.tile([2, PS_CHUNK], f32, tag="sc")
                        nc.scalar.copy(sc[:], ps[:])
                        nc.sync.dma_start(out_d[row : row + 1, :, :], sc[:])

    nc.compile()
    _BUILT[key] = nc
    return nc


def wrap_idx2(ix):
    """Padded edge indices [E_DEV] -> int16 [128, E_DEV/32].

    Tile t's edges [t*TILE_E, (t+1)*TILE_E) split into stream A (first N_IDX)
    on partition rows 0-63 and stream B on rows 64-127; within a stream,
    index j sits at (partition j%16, col t*(N_IDX/16) + j//16), replicated
    across the 4 GPSIMD core groups of that half."""
    x = ix.astype(np.int16).reshape(N_TILES, 2, N_IDX // 16, 16)
    w = x.transpose(1, 3, 0, 2).reshape(2, 16, -1)  # [2, 16, E_DEV/32]
    out = np.empty((128, E_DEV // 32), np.int16)
    out[0:64] = np.tile(w[0], (4, 1))
    out[64:128] = np.tile(w[1], (4, 1))
    return out


def prep_core(src_k, dst_k):
    s = np.zeros(E_DEV, np.int64)
    d = np.zeros(E_DEV, np.int64)
    s[: len(src_k)] = src_k
    d[: len(dst_k)] = dst_k
    return wrap_idx2(s), wrap_idx2(d)


def pack_h(h):
    """h [10000, 128] f32 -> packed-pair fp16 table [128, 10000, 2]."""
    hp = h.astype(np.float16).reshape(N_NODES, 64, 2).transpose(1, 0, 2)
    return np.ascontiguousarray(np.concatenate([hp, hp], axis=0))


def unscramble(sc):
    """Device scores [20, 2, 2048] f32 -> per-core edge-ordered [E_DEV]."""
    return (
        sc.reshape(N_TILES, N_IDX // PS_CHUNK, 2, PS_CHUNK)
        .transpose(0, 2, 1, 3)
        .reshape(E_DEV)
    )


def kernel(h, src, dst):
    hT2 = pack_h(np.asarray(h, dtype=np.float32))
    src = np.asarray(src).astype(np.int64)
    dst = np.asarray(dst).astype(np.int64)

    nc = build()
    in_maps = []
    for k in range(N_CORES):
        sl = slice(k * E_PER, (k + 1) * E_PER)
        srcw, dstw = prep_core(src[sl], dst[sl])
        in_maps.append({"hT2": hT2, "srcw": srcw, "dstw": dstw})
    res = run_bass_kernel_spmd(nc, in_maps, list(range(N_CORES)))

    out = np.empty(N_EDGES, np.float32)
    for k in range(N_CORES):
        out[k * E_PER : (k + 1) * E_PER] = unscramble(res.results[k]["scores"])[
            :E_PER
        ]
    return out.reshape(N_EDGES, 1)
